# revision 12
# baseline (speedup 1.0000x reference)
"""AttentionBlock (GroupNorm -> 1x1 qkv -> 4-head attention over 64x64 -> proj -> residual)
distributed over 8 Trainium2 NeuronCores.

Sharding: 8 shards = batch(4) x query-half(2). Each core receives the full
[256, 4096] feature map of its batch element (columns rolled so its own query
half is always columns 0:2048 -> identical SPMD graph on every core), computes
GroupNorm + k/v for all 4096 positions and q for its 2048 queries, runs
flash-style attention (scores kept transposed [j, i]; softmax denominator via a
ones-row folded into the v^T matmul; no max subtraction -- scores ~ N(0,1)),
then proj + residual for its query half. No collectives.
"""

import sys

sys.path.insert(0, "/opt/trn_rl_repo")

import numpy as np
import ml_dtypes

import concourse.bass as bass
import concourse.tile as tile
from concourse import bacc, mybir

# Problem geometry (hardcoded per harness contract)
B, C, H, W = 4, 256, 64, 64
N = H * W              # 4096 spatial positions
HEADS = 4
D = C // HEADS         # 64
GROUPS = 8
EPS = 1e-5
NCORES = 8
NI = N // 2            # 2048 queries per core
IB = 1024              # i-block (queries per psum accumulation block)
JC = 128               # j-chunk (key positions per scores matmul)
NJ = N // JC           # 32 j-chunks

F32 = mybir.dt.float32
BF16 = mybir.dt.bfloat16

_CACHE = {}


def _build_nc():
    nc = bacc.Bacc("TRN2", target_bir_lowering=False, debug=False,
                   num_devices=NCORES)

    x_ext = nc.declare_dram_parameter("x", [C, N], F32, isOutput=False)
    wqkT_ext = nc.declare_dram_parameter("wqkT", [C, 2 * C], BF16, isOutput=False)
    wvT_ext = nc.declare_dram_parameter("wvT", [C, C], BF16, isOutput=False)
    wpT_ext = nc.declare_dram_parameter("wpT", [C, C], BF16, isOutput=False)
    qkb_ext = nc.declare_dram_parameter("qkb", [128, 4], F32, isOutput=False)
    pb_ext = nc.declare_dram_parameter("pb", [128, 2], F32, isOutput=False)
    gnw_ext = nc.declare_dram_parameter("gnw", [128, 2], F32, isOutput=False)
    gnb_ext = nc.declare_dram_parameter("gnb", [128, 2], F32, isOutput=False)
    oneh_ext = nc.declare_dram_parameter("oneh", [C, GROUPS], F32, isOutput=False)
    onehT_ext = nc.declare_dram_parameter("onehT", [GROUPS, C], F32, isOutput=False)
    out_ext = nc.declare_dram_parameter("out", [C, NI], F32, isOutput=True)

    with tile.TileContext(nc) as tc:
        with (
            tc.tile_pool(name="persist", bufs=1) as per,
            tc.tile_pool(name="xp", bufs=3) as xp,
            tc.tile_pool(name="ep", bufs=2) as ep,
            tc.tile_pool(name="yp", bufs=2) as yp,
            tc.tile_pool(name="dp", bufs=2, space="DRAM") as dp,
            tc.tile_pool(name="ps", bufs=3, space="PSUM") as ps,
            tc.tile_pool(name="pv", bufs=1, space="PSUM") as pvp,
        ):
            # ---- persistent SBUF tensors ----
            x_sb = [per.tile([128, N], F32, tag=f"x{t}", name=f"x{t}") for t in range(2)]
            xn_sb = [per.tile([128, N], BF16, tag=f"xn{t}", name=f"xn{t}") for t in range(2)]
            q_sb = [per.tile([128, NI], BF16, tag=f"q{t}", name=f"q{t}") for t in range(2)]
            k_sb = [per.tile([128, N], BF16, tag=f"k{t}", name=f"k{t}") for t in range(2)]
            # v^T per j-chunk + ones column per head: [part=s%128, jchunk, head, 64+1]
            vton = per.tile([128, NJ, HEADS, D + 1], BF16, tag="vton")
            att_sb = [per.tile([128, NI], BF16, tag=f"att{t}", name=f"att{t}") for t in range(2)]
            wqkT_sb = [per.tile([128, 2 * C], BF16, tag=f"wqk{t}", name=f"wqk{t}") for t in range(2)]
            wvT_sb = [per.tile([128, C], BF16, tag=f"wv{t}", name=f"wv{t}") for t in range(2)]
            wpT_sb = [per.tile([128, C], BF16, tag=f"wp{t}", name=f"wp{t}") for t in range(2)]
            qkb_sb = per.tile([128, 4], F32, tag="qkb")
            pb_sb = per.tile([128, 2], F32, tag="pb")
            gnw_sb = per.tile([128, 2], F32, tag="gnw")
            gnb_sb = per.tile([128, 2], F32, tag="gnb")
            oneh_sb = [per.tile([128, GROUPS], F32, tag=f"oneh{t}", name=f"oneh{t}") for t in range(2)]
            onehT_sb = per.tile([GROUPS, C], F32, tag="onehT")
            eps_sb = per.tile([GROUPS, 1], F32, tag="eps")
            ab_sb = [per.tile([128, 2], F32, tag=f"ab{t}", name=f"ab{t}") for t in range(2)]
            gst_sb = per.tile([GROUPS, 4], F32, tag="gst")

            nc.vector.memset(eps_sb[:], EPS)
            nc.vector.memset(vton[:, :, :, D : D + 1], 1.0)

            # ---- weight / small-input DMAs ----
            for t in range(2):
                cs = slice(t * 128, (t + 1) * 128)
                nc.sync.dma_start(out=wqkT_sb[t][:], in_=wqkT_ext[cs, :])
                nc.sync.dma_start(out=wvT_sb[t][:], in_=wvT_ext[cs, :])
                nc.sync.dma_start(out=wpT_sb[t][:], in_=wpT_ext[cs, :])
                nc.sync.dma_start(out=oneh_sb[t][:], in_=oneh_ext[cs, :])
            nc.sync.dma_start(out=qkb_sb[:], in_=qkb_ext[:])
            nc.sync.dma_start(out=pb_sb[:], in_=pb_ext[:])
            nc.sync.dma_start(out=gnw_sb[:], in_=gnw_ext[:])
            nc.sync.dma_start(out=gnb_sb[:], in_=gnb_ext[:])
            nc.sync.dma_start(out=onehT_sb[:], in_=onehT_ext[:])

            # ---- GroupNorm statistics (bn_stats over 512-chunks, 8 per tile) ----
            stats = [per.tile([128, 8, 6], F32, tag=f"st{t}", name=f"st{t}") for t in range(2)]
            mv = [per.tile([128, 4], F32, tag=f"mv{t}", name=f"mv{t}") for t in range(2)]
            dma_engs = [nc.sync, nc.gpsimd, nc.scalar, nc.sync]
            for ch in range(4):
                for t in range(2):
                    cs = slice(t * 128, (t + 1) * 128)
                    dma_engs[(ch * 2 + t) % 4].dma_start(
                        out=x_sb[t][:, ch * IB : (ch + 1) * IB],
                        in_=x_ext[cs, ch * IB : (ch + 1) * IB],
                    )
                    for s in range(2):
                        sub = ch * 2 + s
                        nc.vector.bn_stats(
                            out=stats[t][:, sub, :],
                            in_=x_sb[t][:, sub * 512 : (sub + 1) * 512],
                        )
            for t in range(2):
                # mv layout: 0=mean, 1=var, 2=mean (copy), 3=E[x^2]; matmul rhs = cols 2:4
                nc.vector.bn_aggr(out=mv[t][:, 0:2], in_=stats[t][:])
                nc.vector.tensor_copy(mv[t][:, 2:3], mv[t][:, 0:1])
                nc.vector.tensor_mul(mv[t][:, 3:4], mv[t][:, 0:1], mv[t][:, 0:1])
                nc.vector.tensor_add(mv[t][:, 3:4], mv[t][:, 1:2], mv[t][:, 3:4])

            # group means of (mean, E[x^2]): [8, 2] = sum_t oneh[t].T @ mv[t][:, 0:1|2:3]
            gp = ps.tile([GROUPS, 2], F32, tag="ps")
            for t in range(2):
                nc.tensor.matmul(
                    gp[:], oneh_sb[t][:], mv[t][:, 2:4],
                    start=(t == 0), stop=(t == 1),
                )
            # gst columns: 0=mean_g, 1=rstd_g; scratch 2=var, 3=std
            nc.vector.tensor_copy(gst_sb[:, 0:1], gp[:, 0:1])
            nc.vector.tensor_mul(gst_sb[:, 2:3], gst_sb[:, 0:1], gst_sb[:, 0:1])
            nc.vector.tensor_sub(gst_sb[:, 2:3], gp[:, 1:2], gst_sb[:, 2:3])
            nc.scalar.activation(
                out=gst_sb[:, 3:4], in_=gst_sb[:, 2:3],
                func=mybir.ActivationFunctionType.Sqrt,
                bias=eps_sb[:], scale=1.0,
            )
            nc.vector.reciprocal(gst_sb[:, 1:2], gst_sb[:, 3:4])

            # broadcast (mean_g, rstd_g) back to channels, form per-channel affine
            for t in range(2):
                bc = ps.tile([128, 2], F32, tag="ps")
                nc.tensor.matmul(
                    bc[:], onehT_sb[:, t * 128 : (t + 1) * 128], gst_sb[:, 0:2],
                    start=True, stop=True,
                )
                # a = rstd * gn_w ; b = gn_b - mean * a
                nc.vector.tensor_mul(ab_sb[t][:, 0:1], bc[:, 1:2], gnw_sb[:, t : t + 1])
                nc.vector.tensor_mul(ab_sb[t][:, 1:2], bc[:, 0:1], ab_sb[t][:, 0:1])
                nc.vector.tensor_sub(ab_sb[t][:, 1:2], gnb_sb[:, t : t + 1], ab_sb[t][:, 1:2])
                for ch in range(4):
                    cols = slice(ch * IB, (ch + 1) * IB)
                    nc.vector.tensor_scalar(
                        out=xn_sb[t][:, cols], in0=x_sb[t][:, cols],
                        scalar1=ab_sb[t][:, 0:1], scalar2=ab_sb[t][:, 1:2],
                        op0=mybir.AluOpType.mult, op1=mybir.AluOpType.add,
                    )

            # ---- emission helpers (PE executes in emission order: start
            # attention as early as possible, fill qkv/vT/proj into its shadow) ----
            def qkv_tiles(ot):
                # ot 0,1 = q o-tiles; 2,3 = k o-tiles (wqkT cols 0:256 q, 256:512 k)
                dest = q_sb[ot] if ot < 2 else k_sb[ot - 2]
                ncols = NI if ot < 2 else N
                wcols = slice(ot * 128, (ot + 1) * 128)
                for nb in range(ncols // IB):
                    pp = ps.tile([128, IB], F32, tag="ps", name=f"qkv{ot}_{nb}")
                    for cc in range(2):
                        for nh in range(2):
                            nsl = slice(nb * IB + nh * 512, nb * IB + (nh + 1) * 512)
                            psl = slice(nh * 512, (nh + 1) * 512)
                            nc.tensor.matmul(
                                pp[:, psl], wqkT_sb[cc][:, wcols], xn_sb[cc][:, nsl],
                                start=(cc == 0), stop=(cc == 1),
                            )
                    nc.vector.tensor_scalar_add(
                        out=dest[:, nb * IB : (nb + 1) * IB], in0=pp[:],
                        scalar1=qkb_sb[:, ot : ot + 1],
                    )

            def vt_chunk(j):
                # v^T for s-chunk j: [s128, 256] = xn_chunk.T @ wvT
                pj = ps.tile([128, C], F32, tag="ps", name=f"vt{j}")
                jsl = slice(j * JC, (j + 1) * JC)
                for cc in range(2):
                    nc.tensor.matmul(
                        pj[:], xn_sb[cc][:, jsl], wvT_sb[cc][:],
                        start=(cc == 0), stop=(cc == 1),
                    )
                nc.vector.tensor_copy(
                    out=vton[:, j, :, 0:D],
                    in_=pj[:].rearrange("p (h d) -> p h d", h=HEADS),
                )

            def attn_head(ib, h, with_vt, mid_cb=None):
                ht, hp = divmod(h, 2)
                prow = slice(hp * D, (hp + 1) * D)
                isl = slice(ib * IB, (ib + 1) * IB)
                pv = pvp.tile([D + 1, IB], F32, tag="pv", name=f"pv{ib}_{h}")
                for j in range(NJ):
                    jsl = slice(j * JC, (j + 1) * JC)
                    sc = ps.tile([128, IB], F32, tag="ps", name=f"sc{ib}_{h}_{j}")
                    et = xp.tile([128, IB], BF16, tag="et", name=f"et{ib}_{h}_{j}")
                    for nh in range(2):
                        psl = slice(nh * 512, (nh + 1) * 512)
                        qsl = slice(ib * IB + nh * 512, ib * IB + (nh + 1) * 512)
                        nc.tensor.matmul(
                            sc[:, psl], k_sb[ht][prow, jsl], q_sb[ht][prow, qsl],
                            start=True, stop=True,
                        )
                    nc.scalar.activation(
                        out=et[:], in_=sc[:],
                        func=mybir.ActivationFunctionType.Exp,
                        scale=float(D) ** -0.5,
                    )
                    if with_vt:
                        vt_chunk(j)
                    if mid_cb is not None and j == 8:
                        mid_cb()
                    for nh in range(2):
                        psl = slice(nh * 512, (nh + 1) * 512)
                        nc.tensor.matmul(
                            pv[:, psl], vton[:, j, h, :], et[:, psl],
                            start=(j == 0), stop=(j == NJ - 1),
                        )
                # copy pv out of PSUM immediately (frees pv for the next head);
                # normalization runs off the critical path: denominator row is
                # broadcast across 64 partitions via a DRAM round trip.
                pvs = ep.tile([D + 1, IB], F32, tag="pvs", name=f"pvs{ib}_{h}")
                nc.vector.tensor_copy(pvs[:], pv[:])
                dent = dp.tile([1, IB], F32, tag="dent", name=f"den{ib}_{h}")
                nc.sync.dma_start(out=dent[:], in_=pvs[D : D + 1, :])
                rbs = ep.tile([D, IB], F32, tag="rbs", name=f"rbs{ib}_{h}")
                for nh in range(2):
                    hsl = slice(nh * 512, (nh + 1) * 512)
                    half = dent[0:1, hsl]
                    dbc = bass.AP(
                        tensor=half.tensor, offset=half.offset,
                        ap=[[0, D]] + [list(a) for a in half.ap[1:]],
                    )
                    nc.sync.dma_start(out=rbs[:, hsl], in_=dbc)
                    nc.vector.reciprocal_approx_fast(out=rbs[:, hsl], in_=rbs[:, hsl])
                    nc.vector.tensor_mul(
                        att_sb[ht][prow, ib * IB + nh * 512 : ib * IB + (nh + 1) * 512],
                        pvs[0:D, hsl], rbs[:, hsl],
                    )

            def proj_part(ib, cc, ypart_tiles):
                # proj c-chunk cc (attention channels of heads 2cc, 2cc+1)
                isl = slice(ib * IB, (ib + 1) * IB)
                for ot in range(2):
                    pp = ps.tile([128, IB], F32, tag="ps", name=f"pj{ib}_{cc}_{ot}")
                    wcols = slice(ot * 128, (ot + 1) * 128)
                    for nh in range(2):
                        psl = slice(nh * 512, (nh + 1) * 512)
                        asl = slice(ib * IB + nh * 512, ib * IB + (nh + 1) * 512)
                        nc.tensor.matmul(
                            pp[:, psl], wpT_sb[cc][:, wcols], att_sb[cc][:, asl],
                            start=True, stop=True,
                        )
                    if cc == 0:
                        yt = yp.tile([128, IB], F32, tag=f"ypart{ot}",
                                     name=f"ypart{ib}_{ot}")
                        nc.vector.tensor_scalar_add(
                            out=yt[:], in0=pp[:], scalar1=pb_sb[:, ot : ot + 1]
                        )
                        ypart_tiles.append(yt)
                    else:
                        y_sb = yp.tile([128, IB], F32, tag="y", name=f"y{ib}_{ot}")
                        nc.vector.tensor_add(y_sb[:], ypart_tiles[ot][:], x_sb[ot][:, isl])
                        nc.vector.tensor_add(y_sb[:], y_sb[:], pp[:])
                        nc.sync.dma_start(
                            out=out_ext[ot * 128 : (ot + 1) * 128, isl], in_=y_sb[:]
                        )

            # ---- schedule ----
            # proj matmuls are emitted 8 j-chunks into a LATER head's loop so
            # the in-order PE stream never stalls waiting on an epilogue
            # normalization chain (a stall re-throttles the PE clock to 1/2).
            qkv_tiles(0)   # q heads 0,1
            qkv_tiles(2)   # k heads 0,1
            yparts = {}
            for ib in range(NI // IB):
                yparts[ib] = []
                for h in range(HEADS):
                    if ib > 0 and h == 0:
                        mid = (lambda p=ib - 1: proj_part(p, 1, yparts[p]))
                    elif h == 3:
                        mid = (lambda p=ib: proj_part(p, 0, yparts[p]))
                    else:
                        mid = None
                    attn_head(ib, h, with_vt=(ib == 0 and h == 0), mid_cb=mid)
                    if ib == 0 and h == 0:
                        qkv_tiles(1)   # q heads 2,3
                        qkv_tiles(3)   # k heads 2,3
            proj_part(NI // IB - 1, 1, yparts[NI // IB - 1])

    nc.compile()
    return nc


def _prep_in_maps(x, gn_w, gn_b, qkv_w, qkv_b, proj_w, proj_b):
    x = np.ascontiguousarray(np.asarray(x, np.float32)).reshape(B, C, N)
    qkv_w = np.asarray(qkv_w, np.float32)
    qkv_b = np.asarray(qkv_b, np.float32)
    proj_w = np.asarray(proj_w, np.float32)
    proj_b = np.asarray(proj_b, np.float32)
    gn_w = np.asarray(gn_w, np.float32)
    gn_b = np.asarray(gn_b, np.float32)

    bf = ml_dtypes.bfloat16
    wqkT = np.ascontiguousarray(qkv_w[: 2 * C].T).astype(bf)        # [256, 512]
    wvT = np.ascontiguousarray(qkv_w[2 * C :].T).astype(bf)         # [256, 256]
    wpT = np.ascontiguousarray(proj_w.T).astype(bf)                 # [256, 256]
    qkb = np.ascontiguousarray(qkv_b[: 2 * C].reshape(4, 128).T)    # [128, 4]
    # fold v-bias through proj: proj(att + vb) = proj(att) + proj_w @ vb
    pb_eff = proj_b + proj_w.astype(np.float64) @ qkv_b[2 * C :].astype(np.float64)
    pb = np.ascontiguousarray(pb_eff.astype(np.float32).reshape(2, 128).T)
    gnw2 = np.ascontiguousarray(gn_w.reshape(2, 128).T)
    gnb2 = np.ascontiguousarray(gn_b.reshape(2, 128).T)
    cidx = np.arange(C)
    oneh = (cidx[:, None] // 32 == np.arange(GROUPS)[None, :]).astype(np.float32) / 32.0
    onehT = np.ascontiguousarray(oneh.T * 32.0)

    shared = {
        "wqkT": wqkT, "wvT": wvT, "wpT": wpT, "qkb": qkb, "pb": pb,
        "gnw": gnw2, "gnb": gnb2, "oneh": oneh, "onehT": onehT,
    }
    in_maps = []
    for core in range(NCORES):
        bi, half = divmod(core, 2)
        xb = x[bi]
        if half:
            xs = np.ascontiguousarray(np.concatenate([xb[:, NI:], xb[:, :NI]], axis=1))
        else:
            xs = xb
        in_maps.append({"x": xs, **shared})
    return in_maps


def _assemble(results):
    y = np.empty((B, C, N), np.float32)
    for core in range(NCORES):
        bi, half = divmod(core, 2)
        y[bi][:, half * NI : (half + 1) * NI] = results[core]["out"]
    return y.reshape(B, C, H, W)


def kernel(x, gn_w, gn_b, qkv_w, qkv_b, proj_w, proj_b):
    from concourse.bass_utils import run_bass_kernel_spmd

    if "nc" not in _CACHE:
        _CACHE["nc"] = _build_nc()
    nc = _CACHE["nc"]
    in_maps = _prep_in_maps(x, gn_w, gn_b, qkv_w, qkv_b, proj_w, proj_b)
    res = run_bass_kernel_spmd(nc, in_maps, core_ids=list(range(NCORES)))
    return _assemble(res.results)


# revision 14
# speedup vs baseline: 1.3810x; 1.3810x over previous
"""AttentionBlock (GroupNorm -> 1x1 qkv -> 4-head attention over 64x64 -> proj -> residual)
distributed over 8 Trainium2 NeuronCores.

Sharding: 8 shards = batch(4) x query-half(2). Each core receives the full
[256, 4096] feature map of its batch element (columns rolled so its own query
half is always columns 0:2048 -> identical SPMD graph on every core), computes
GroupNorm + k/v for all 4096 positions and q for its 2048 queries, runs
flash-style attention (scores kept transposed [j, i]; softmax denominator via a
ones-row folded into the v^T matmul; no max subtraction -- scores ~ N(0,1)),
then proj + residual for its query half. No collectives.
"""

import sys

sys.path.insert(0, "/opt/trn_rl_repo")

import numpy as np
import ml_dtypes

import concourse.bass as bass
import concourse.tile as tile
from concourse import bacc, mybir

# Problem geometry (hardcoded per harness contract)
B, C, H, W = 4, 256, 64, 64
N = H * W              # 4096 spatial positions
HEADS = 4
D = C // HEADS         # 64
GROUPS = 8
EPS = 1e-5
NCORES = 8
NI = N // 2            # 2048 queries per core
IB = 1024              # i-block (queries per psum accumulation block)
JC = 128               # j-chunk (key positions per scores matmul)
NJ = N // JC           # 32 j-chunks

F32 = mybir.dt.float32
BF16 = mybir.dt.bfloat16

_CACHE = {}


def _build_nc():
    nc = bacc.Bacc("TRN2", target_bir_lowering=False, debug=False,
                   num_devices=NCORES)

    x_ext = nc.declare_dram_parameter("x", [C, N], F32, isOutput=False)
    wqkT_ext = nc.declare_dram_parameter("wqkT", [C, 2 * C], BF16, isOutput=False)
    wvT_ext = nc.declare_dram_parameter("wvT", [C, C], BF16, isOutput=False)
    wpT_ext = nc.declare_dram_parameter("wpT", [C, C], BF16, isOutput=False)
    qkb_ext = nc.declare_dram_parameter("qkb", [128, 4], F32, isOutput=False)
    pb_ext = nc.declare_dram_parameter("pb", [128, 2], F32, isOutput=False)
    gnw_ext = nc.declare_dram_parameter("gnw", [128, 2], F32, isOutput=False)
    gnb_ext = nc.declare_dram_parameter("gnb", [128, 2], F32, isOutput=False)
    oneh_ext = nc.declare_dram_parameter("oneh", [C, GROUPS], F32, isOutput=False)
    onehT_ext = nc.declare_dram_parameter("onehT", [GROUPS, C], F32, isOutput=False)
    out_ext = nc.declare_dram_parameter("out", [C, NI], F32, isOutput=True)

    with tile.TileContext(nc) as tc:
        with (
            tc.tile_pool(name="persist", bufs=1) as per,
            tc.tile_pool(name="xp", bufs=3) as xp,
            tc.tile_pool(name="ep", bufs=2) as ep,
            tc.tile_pool(name="yp", bufs=2) as yp,
            tc.tile_pool(name="dp", bufs=2, space="DRAM") as dp,
            tc.tile_pool(name="ps", bufs=3, space="PSUM") as ps,
            tc.tile_pool(name="pv", bufs=1, space="PSUM") as pvp,
        ):
            # ---- persistent SBUF tensors ----
            x_sb = [per.tile([128, N], F32, tag=f"x{t}", name=f"x{t}") for t in range(2)]
            xn_sb = [per.tile([128, N], BF16, tag=f"xn{t}", name=f"xn{t}") for t in range(2)]
            q_sb = [per.tile([128, NI], BF16, tag=f"q{t}", name=f"q{t}") for t in range(2)]
            k_sb = [per.tile([128, N], BF16, tag=f"k{t}", name=f"k{t}") for t in range(2)]
            # v^T per j-chunk + ones column per head: [part=s%128, jchunk, head, 64+1]
            vton = per.tile([128, NJ, HEADS, D + 1], BF16, tag="vton")
            att_sb = [per.tile([128, NI], BF16, tag=f"att{t}", name=f"att{t}") for t in range(2)]
            wqkT_sb = [per.tile([128, 2 * C], BF16, tag=f"wqk{t}", name=f"wqk{t}") for t in range(2)]
            wvT_sb = [per.tile([128, C], BF16, tag=f"wv{t}", name=f"wv{t}") for t in range(2)]
            wpT_sb = [per.tile([128, C], BF16, tag=f"wp{t}", name=f"wp{t}") for t in range(2)]
            qkb_sb = per.tile([128, 4], F32, tag="qkb")
            pb_sb = per.tile([128, 2], F32, tag="pb")
            gnw_sb = per.tile([128, 2], F32, tag="gnw")
            gnb_sb = per.tile([128, 2], F32, tag="gnb")
            oneh_sb = [per.tile([128, GROUPS], F32, tag=f"oneh{t}", name=f"oneh{t}") for t in range(2)]
            onehT_sb = per.tile([GROUPS, C], F32, tag="onehT")
            eps_sb = per.tile([GROUPS, 1], F32, tag="eps")
            ab_sb = [per.tile([128, 2], F32, tag=f"ab{t}", name=f"ab{t}") for t in range(2)]
            gst_sb = per.tile([GROUPS, 4], F32, tag="gst")

            nc.vector.memset(eps_sb[:], EPS)
            nc.vector.memset(vton[:, :, :, D : D + 1], 1.0)

            # ---- weight / small-input DMAs ----
            for t in range(2):
                cs = slice(t * 128, (t + 1) * 128)
                nc.sync.dma_start(out=wqkT_sb[t][:], in_=wqkT_ext[cs, :])
                nc.sync.dma_start(out=wvT_sb[t][:], in_=wvT_ext[cs, :])
                nc.sync.dma_start(out=wpT_sb[t][:], in_=wpT_ext[cs, :])
                nc.sync.dma_start(out=oneh_sb[t][:], in_=oneh_ext[cs, :])
            nc.sync.dma_start(out=qkb_sb[:], in_=qkb_ext[:])
            nc.sync.dma_start(out=pb_sb[:], in_=pb_ext[:])
            nc.sync.dma_start(out=gnw_sb[:], in_=gnw_ext[:])
            nc.sync.dma_start(out=gnb_sb[:], in_=gnb_ext[:])
            nc.sync.dma_start(out=onehT_sb[:], in_=onehT_ext[:])

            # ---- GroupNorm statistics (bn_stats over 512-chunks, 8 per tile) ----
            stats = [per.tile([128, 8, 6], F32, tag=f"st{t}", name=f"st{t}") for t in range(2)]
            mv = [per.tile([128, 4], F32, tag=f"mv{t}", name=f"mv{t}") for t in range(2)]
            for t in range(2):
                cs = slice(t * 128, (t + 1) * 128)
                dma_eng = nc.sync if t == 0 else nc.gpsimd
                for ch in range(4):
                    dma_eng.dma_start(
                        out=x_sb[t][:, ch * IB : (ch + 1) * IB],
                        in_=x_ext[cs, ch * IB : (ch + 1) * IB],
                    )
                    for s in range(2):
                        sub = ch * 2 + s
                        nc.vector.bn_stats(
                            out=stats[t][:, sub, :],
                            in_=x_sb[t][:, sub * 512 : (sub + 1) * 512],
                        )
                # mv layout: 0=mean, 1=var, 2=mean (copy), 3=E[x^2]; matmul rhs = cols 2:4
                nc.vector.bn_aggr(out=mv[t][:, 0:2], in_=stats[t][:])
                nc.vector.tensor_copy(mv[t][:, 2:3], mv[t][:, 0:1])
                nc.vector.tensor_mul(mv[t][:, 3:4], mv[t][:, 0:1], mv[t][:, 0:1])
                nc.vector.tensor_add(mv[t][:, 3:4], mv[t][:, 1:2], mv[t][:, 3:4])

            # group means of (mean, E[x^2]): [8, 2] = sum_t oneh[t].T @ mv[t][:, 0:1|2:3]
            gp = ps.tile([GROUPS, 2], F32, tag="ps")
            for t in range(2):
                nc.tensor.matmul(
                    gp[:], oneh_sb[t][:], mv[t][:, 2:4],
                    start=(t == 0), stop=(t == 1),
                )
            # gst columns: 0=mean_g, 1=rstd_g; scratch 2=var, 3=std
            nc.vector.tensor_copy(gst_sb[:, 0:1], gp[:, 0:1])
            nc.vector.tensor_mul(gst_sb[:, 2:3], gst_sb[:, 0:1], gst_sb[:, 0:1])
            nc.vector.tensor_sub(gst_sb[:, 2:3], gp[:, 1:2], gst_sb[:, 2:3])
            # rstd = exp(-0.5*ln(var+eps)): Ln and Exp share one ACT table set
            # (natural_log_exp), so the softmax exps never pay a table switch.
            nc.scalar.activation(
                out=gst_sb[:, 3:4], in_=gst_sb[:, 2:3],
                func=mybir.ActivationFunctionType.Ln,
                bias=eps_sb[:], scale=1.0,
            )
            nc.vector.tensor_scalar_mul(
                out=gst_sb[:, 3:4], in0=gst_sb[:, 3:4], scalar1=-0.5
            )
            nc.scalar.activation(
                out=gst_sb[:, 1:2], in_=gst_sb[:, 3:4],
                func=mybir.ActivationFunctionType.Exp, scale=1.0,
            )

            # broadcast (mean_g, rstd_g) back to channels, form per-channel affine
            for t in range(2):
                bc = ps.tile([128, 2], F32, tag="ps")
                nc.tensor.matmul(
                    bc[:], onehT_sb[:, t * 128 : (t + 1) * 128], gst_sb[:, 0:2],
                    start=True, stop=True,
                )
                # a = rstd * gn_w ; b = gn_b - mean * a
                nc.vector.tensor_mul(ab_sb[t][:, 0:1], bc[:, 1:2], gnw_sb[:, t : t + 1])
                nc.vector.tensor_mul(ab_sb[t][:, 1:2], bc[:, 0:1], ab_sb[t][:, 0:1])
                nc.vector.tensor_sub(ab_sb[t][:, 1:2], gnb_sb[:, t : t + 1], ab_sb[t][:, 1:2])
                for ch in range(4):
                    cols = slice(ch * IB, (ch + 1) * IB)
                    nc.vector.tensor_scalar(
                        out=xn_sb[t][:, cols], in0=x_sb[t][:, cols],
                        scalar1=ab_sb[t][:, 0:1], scalar2=ab_sb[t][:, 1:2],
                        op0=mybir.AluOpType.mult, op1=mybir.AluOpType.add,
                    )

            # ---- emission helpers (PE executes in emission order: start
            # attention as early as possible, fill qkv/vT/proj into its shadow) ----
            def qkv_tiles(ot, blocks=None):
                # ot 0,1 = q o-tiles; 2,3 = k o-tiles (wqkT cols 0:256 q, 256:512 k)
                dest = q_sb[ot] if ot < 2 else k_sb[ot - 2]
                ncols = NI if ot < 2 else N
                wcols = slice(ot * 128, (ot + 1) * 128)
                for nb in (range(ncols // IB) if blocks is None else blocks):
                    pp = ps.tile([128, IB], F32, tag="ps", name=f"qkv{ot}_{nb}")
                    for cc in range(2):
                        for nh in range(2):
                            nsl = slice(nb * IB + nh * 512, nb * IB + (nh + 1) * 512)
                            psl = slice(nh * 512, (nh + 1) * 512)
                            nc.tensor.matmul(
                                pp[:, psl], wqkT_sb[cc][:, wcols], xn_sb[cc][:, nsl],
                                start=(cc == 0), stop=(cc == 1),
                            )
                    nc.vector.tensor_scalar_add(
                        out=dest[:, nb * IB : (nb + 1) * IB], in0=pp[:],
                        scalar1=qkb_sb[:, ot : ot + 1],
                    )

            def vt_chunk(j):
                # v^T for s-chunk j: [s128, 256] = xn_chunk.T @ wvT
                pj = ps.tile([128, C], F32, tag="ps", name=f"vt{j}")
                jsl = slice(j * JC, (j + 1) * JC)
                for cc in range(2):
                    nc.tensor.matmul(
                        pj[:], xn_sb[cc][:, jsl], wvT_sb[cc][:],
                        start=(cc == 0), stop=(cc == 1),
                    )
                nc.vector.tensor_copy(
                    out=vton[:, j, :, 0:D],
                    in_=pj[:].rearrange("p (h d) -> p h d", h=HEADS),
                )

            def attn_head(ib, h, with_vt, mid_cb=None):
                ht, hp = divmod(h, 2)
                prow = slice(hp * D, (hp + 1) * D)
                isl = slice(ib * IB, (ib + 1) * IB)
                pv = pvp.tile([D + 1, IB], F32, tag="pv", name=f"pv{ib}_{h}")
                for j in range(NJ):
                    jsl = slice(j * JC, (j + 1) * JC)
                    sc = ps.tile([128, IB], F32, tag="ps", name=f"sc{ib}_{h}_{j}")
                    et = xp.tile([128, IB], BF16, tag="et", name=f"et{ib}_{h}_{j}")
                    for nh in range(2):
                        psl = slice(nh * 512, (nh + 1) * 512)
                        qsl = slice(ib * IB + nh * 512, ib * IB + (nh + 1) * 512)
                        nc.tensor.matmul(
                            sc[:, psl], k_sb[ht][prow, jsl], q_sb[ht][prow, qsl],
                            start=True, stop=True,
                        )
                    nc.scalar.activation(
                        out=et[:], in_=sc[:],
                        func=mybir.ActivationFunctionType.Exp,
                        scale=float(D) ** -0.5,
                    )
                    if with_vt:
                        vt_chunk(j)
                    if mid_cb is not None:
                        mid_cb(j)
                    for nh in range(2):
                        psl = slice(nh * 512, (nh + 1) * 512)
                        nc.tensor.matmul(
                            pv[:, psl], vton[:, j, h, :], et[:, psl],
                            start=(j == 0), stop=(j == NJ - 1),
                        )
                # copy pv out of PSUM immediately (frees pv for the next head);
                # normalization runs off the critical path: denominator row is
                # broadcast across 64 partitions via a DRAM round trip.
                pvs = ep.tile([D + 1, IB], F32, tag="pvs", name=f"pvs{ib}_{h}")
                nc.vector.tensor_copy(pvs[:], pv[:])
                dent = dp.tile([1, IB], F32, tag="dent", name=f"den{ib}_{h}")
                nc.sync.dma_start(out=dent[:], in_=pvs[D : D + 1, :])
                rbs = ep.tile([D, IB], F32, tag="rbs", name=f"rbs{ib}_{h}")
                dbc = bass.AP(
                    tensor=dent.tensor, offset=dent.offset,
                    ap=[[0, D]] + [list(a) for a in dent.ap[1:]],
                )
                nc.sync.dma_start(out=rbs[:], in_=dbc)
                nc.vector.reciprocal_approx_fast(out=rbs[:], in_=rbs[:])
                nc.vector.tensor_mul(att_sb[ht][prow, isl], pvs[0:D, :], rbs[:])

            def proj_part(ib, cc, ypart_tiles):
                # proj c-chunk cc (attention channels of heads 2cc, 2cc+1)
                isl = slice(ib * IB, (ib + 1) * IB)
                for ot in range(2):
                    pp = ps.tile([128, IB], F32, tag="ps", name=f"pj{ib}_{cc}_{ot}")
                    wcols = slice(ot * 128, (ot + 1) * 128)
                    for nh in range(2):
                        psl = slice(nh * 512, (nh + 1) * 512)
                        asl = slice(ib * IB + nh * 512, ib * IB + (nh + 1) * 512)
                        nc.tensor.matmul(
                            pp[:, psl], wpT_sb[cc][:, wcols], att_sb[cc][:, asl],
                            start=True, stop=True,
                        )
                    if cc == 0:
                        yt = yp.tile([128, IB], F32, tag=f"ypart{ot}",
                                     name=f"ypart{ib}_{ot}")
                        nc.vector.tensor_scalar_add(
                            out=yt[:], in0=pp[:], scalar1=pb_sb[:, ot : ot + 1]
                        )
                        ypart_tiles.append(yt)
                    else:
                        y_sb = yp.tile([128, IB], F32, tag="y", name=f"y{ib}_{ot}")
                        nc.vector.tensor_add(y_sb[:], ypart_tiles[ot][:], x_sb[ot][:, isl])
                        nc.vector.tensor_add(y_sb[:], y_sb[:], pp[:])
                        nc.sync.dma_start(
                            out=out_ext[ot * 128 : (ot + 1) * 128, isl], in_=y_sb[:]
                        )

            # ---- schedule ----
            # proj matmuls are emitted 8 j-chunks into a LATER head's loop so
            # the in-order PE stream never stalls waiting on an epilogue
            # normalization chain (a stall re-throttles the PE clock to 1/2).
            qkv_tiles(0)        # q heads 0,1
            qkv_tiles(2, [0])   # k heads 0,1, first j-block only
            yparts = {}
            for ib in range(NI // IB):
                yparts[ib] = []
                for h in range(HEADS):
                    if ib == 0 and h == 0:
                        # k blocks 1-3 stream in 8 j-chunks ahead of first use
                        mid = (lambda j: qkv_tiles(2, [1 + j // 8])
                               if j in (0, 8, 16) else None)
                    elif ib > 0 and h == 0:
                        mid = (lambda j, p=ib - 1: proj_part(p, 1, yparts[p])
                               if j == 8 else None)
                    elif h == 3:
                        mid = (lambda j, p=ib: proj_part(p, 0, yparts[p])
                               if j == 8 else None)
                    else:
                        mid = None
                    attn_head(ib, h, with_vt=(ib == 0 and h == 0), mid_cb=mid)
                    if ib == 0 and h == 0:
                        qkv_tiles(1)   # q heads 2,3
                        qkv_tiles(3)   # k heads 2,3
            proj_part(NI // IB - 1, 1, yparts[NI // IB - 1])

    nc.compile()
    return nc


def _prep_in_maps(x, gn_w, gn_b, qkv_w, qkv_b, proj_w, proj_b):
    x = np.ascontiguousarray(np.asarray(x, np.float32)).reshape(B, C, N)
    qkv_w = np.asarray(qkv_w, np.float32)
    qkv_b = np.asarray(qkv_b, np.float32)
    proj_w = np.asarray(proj_w, np.float32)
    proj_b = np.asarray(proj_b, np.float32)
    gn_w = np.asarray(gn_w, np.float32)
    gn_b = np.asarray(gn_b, np.float32)

    bf = ml_dtypes.bfloat16
    wqkT = np.ascontiguousarray(qkv_w[: 2 * C].T).astype(bf)        # [256, 512]
    wvT = np.ascontiguousarray(qkv_w[2 * C :].T).astype(bf)         # [256, 256]
    wpT = np.ascontiguousarray(proj_w.T).astype(bf)                 # [256, 256]
    qkb = np.ascontiguousarray(qkv_b[: 2 * C].reshape(4, 128).T)    # [128, 4]
    # fold v-bias through proj: proj(att + vb) = proj(att) + proj_w @ vb
    pb_eff = proj_b + proj_w.astype(np.float64) @ qkv_b[2 * C :].astype(np.float64)
    pb = np.ascontiguousarray(pb_eff.astype(np.float32).reshape(2, 128).T)
    gnw2 = np.ascontiguousarray(gn_w.reshape(2, 128).T)
    gnb2 = np.ascontiguousarray(gn_b.reshape(2, 128).T)
    cidx = np.arange(C)
    oneh = (cidx[:, None] // 32 == np.arange(GROUPS)[None, :]).astype(np.float32) / 32.0
    onehT = np.ascontiguousarray(oneh.T * 32.0)

    shared = {
        "wqkT": wqkT, "wvT": wvT, "wpT": wpT, "qkb": qkb, "pb": pb,
        "gnw": gnw2, "gnb": gnb2, "oneh": oneh, "onehT": onehT,
    }
    in_maps = []
    for core in range(NCORES):
        bi, half = divmod(core, 2)
        xb = x[bi]
        if half:
            xs = np.ascontiguousarray(np.concatenate([xb[:, NI:], xb[:, :NI]], axis=1))
        else:
            xs = xb
        in_maps.append({"x": xs, **shared})
    return in_maps


def _assemble(results):
    y = np.empty((B, C, N), np.float32)
    for core in range(NCORES):
        bi, half = divmod(core, 2)
        y[bi][:, half * NI : (half + 1) * NI] = results[core]["out"]
    return y.reshape(B, C, H, W)


def kernel(x, gn_w, gn_b, qkv_w, qkv_b, proj_w, proj_b):
    from concourse.bass_utils import run_bass_kernel_spmd

    if "nc" not in _CACHE:
        _CACHE["nc"] = _build_nc()
    nc = _CACHE["nc"]
    in_maps = _prep_in_maps(x, gn_w, gn_b, qkv_w, qkv_b, proj_w, proj_b)
    res = run_bass_kernel_spmd(nc, in_maps, core_ids=list(range(NCORES)))
    return _assemble(res.results)
